# revision 1
# baseline (speedup 1.0000x reference)
"""ColAttention TRN2 kernel: 8-core data-parallel over batch (2 batches/core).

Math (per batch b, width-column w):
  Q = Wq@x+bq; K = Wk@x+bk; V = Wv@x+bv        (1x1 convs over c)
  S[h,g] = sum_q Q[q,h]K[q,g]; attn = softmax_g(S)
  out = gamma * (V @ attn^T) + x

Device pipeline (all matmuls bf16, fp32 PSUM accumulation):
  host folds bv/gamma*bv via e = gamma*(I+gamma*Wv)^-1 bv:  xb = x+e,
  bq' = bq-Wq@e, bk' = bk-Wk@e  =>  device never touches bv and the
  residual add of xb reproduces the reference exactly (algebra in notes).
  A : Q,K projections (batched over h*w, N=384 chunks)
  B1: per column: S^T[g,h] via MM(lhsT=K_col, rhs=Q_col); exp via ACT
      (no max-subtraction: |S|<~50 so exp stays finite in f32);
      colsum^T[h] via MM(lhsT=expS_col, rhs=ones); 1/colsum on DVE;
      V^T[g,c] via MM(lhsT=x_col, rhs=Wv^T)
  B2: transpose recip^T[h,w-half] -> recipW[w,h] on PE
  B3: bcast tile gamma*r[h] over 128 partitions via K=1 outer-product MM;
      U[c,h] = MM(lhsT=V^T, rhs=expS); final = U*bcast + xb; DMA out.
"""
import sys

sys.path.insert(0, "/opt/trn_rl_repo")

import numpy as np
import ml_dtypes

import concourse.bass as bass
import concourse.bacc as bacc
import concourse.mybir as mybir
import concourse.tile as tile
from concourse.bass_utils import run_bass_kernel_spmd

F32 = mybir.dt.float32
BF16 = mybir.dt.bfloat16
AF = mybir.ActivationFunctionType

P = 128
H = 96          # height = attention sequence length
W = 96          # width  = independent columns
HW = H * W
B_LOC = 2       # batches per core
WH = 48         # columns per w-half
WC = 4          # columns per B-chunk
NCH = WH // WC  # 12 chunks per w-half

# engines for the Q/K projection evacuation, round-robined
_QK_EVAC = ("vector",)
# engine for the final residual add
TTADD_ENGINE = "vector"


def _build():
    nc = bacc.Bacc("TRN2", target_bir_lowering=False, debug=False)

    xb_d = nc.dram_tensor("xb", [B_LOC, 2, P, HW], BF16, kind="ExternalInput")
    cb_d = nc.dram_tensor("cblob", [P, 866], BF16, kind="ExternalInput")
    bb_d = nc.dram_tensor("bblob", [P, 2], F32, kind="ExternalInput")
    out_d = nc.dram_tensor("out", [B_LOC, 2, P, HW], F32, kind="ExternalOutput")

    with tile.TileContext(nc) as tc:
        import contextlib

        ctx = contextlib.ExitStack()
        with ctx:
            consts = ctx.enter_context(tc.tile_pool(name="consts", bufs=1))
            xp = ctx.enter_context(tc.tile_pool(name="xp", bufs=1))
            qkp = ctx.enter_context(tc.tile_pool(name="qkp", bufs=2))
            esp = ctx.enter_context(tc.tile_pool(name="esp", bufs=2))
            vtp = ctx.enter_context(tc.tile_pool(name="vtp", bufs=1))
            rtp = ctx.enter_context(tc.tile_pool(name="rtp", bufs=2))
            fp = ctx.enter_context(tc.tile_pool(name="fp", bufs=1))
            bctp = ctx.enter_context(tc.tile_pool(name="bctp", bufs=3))
            ttp = ctx.enter_context(tc.tile_pool(name="ttp", bufs=3))
            ps = ctx.enter_context(tc.tile_pool(name="ps", bufs=2, space="PSUM"))

            cb_t = consts.tile([P, 866], BF16)
            bb_t = consts.tile([P, 2], F32)
            nc.sync.dma_start(out=cb_t, in_=cb_d.ap())
            nc.sync.dma_start(out=bb_t, in_=bb_d.ap())
            # observers: funnel DMA deps into one engine each (this walrus
            # accepts a single semaphore wait per instruction)
            nc.tensor.ldweights(cb_t[:, 0:128])
            bias_t = consts.tile([P, 2], F32)
            nc.vector.tensor_copy(bias_t, bb_t)
            wq_t = cb_t[:, 0:128].rearrange("p (c m) -> p c m", c=2)
            wk_t = cb_t[:, 128:256].rearrange("p (c m) -> p c m", c=2)
            wvt_t = cb_t[:, 256:768].rearrange("p (c m) -> p c m", c=2)
            bq_t = bias_t[0:64, 0:1]
            bk_t = bias_t[64:128, 0:1]
            gvec_t = bias_t[0:H, 1:2]
            invg_t = cb_t[0:H, 769:770]
            idb_t = cb_t[0:H, 770:866]

            for b in range(B_LOC):
                x_cm = xp.tile([P, 2, HW], BF16, tag="xcm")
                for ci in range(2):
                    nc.sync.dma_start(out=x_cm[:, ci, :], in_=xb_d.ap()[b, ci])
                # h-major views (x_cm holds h-major data in this variant)
                x_cols = [
                    x_cm[:, ci, :].rearrange("p (h w) -> p w h", w=W) for ci in range(2)
                ]
                x_rows = [
                    x_cm[:, ci, :].rearrange("p (h w) -> p h w", w=W) for ci in range(2)
                ]
                f_ts = [fp.tile([P, HW], F32, tag=f"f{ci}", name=f"f{ci}") for ci in range(2)]
                for f in f_ts:
                    nc.vector.memset(f[0:1, 0:1], 0.0)
                f_cols = [f.rearrange("p (h w) -> p w h", w=W) for f in f_ts]

                for half in range(2):
                    # ---- A: Q/K projections for this w-half -------------------
                    q_t = qkp.tile([64, WH * H], BF16, tag="q", bufs=1)
                    k_t = qkp.tile([64, WH * H], BF16, tag="k", bufs=1)
                    ei = 0
                    for (w_l, b_l, o_t) in ((wq_t, bq_t, q_t), (wk_t, bk_t, k_t)):
                        for hc in range(12):  # 8 h-rows x 48 cols = N=384
                            pr = ps.tile([64, 384], F32, tag="s", bufs=1)
                            for ci in range(2):
                                rhs = x_rows[ci][
                                    :, hc * 8 : (hc + 1) * 8, half * WH : (half + 1) * WH
                                ]
                                nc.tensor.matmul(
                                    pr, w_l[:, ci, :], rhs,
                                    start=(ci == 0), stop=(ci == 1),
                                )
                            dst = o_t[:, hc * 384 : (hc + 1) * 384]
                            if _QK_EVAC[ei % len(_QK_EVAC)] == "act":
                                nc.scalar.activation(
                                    out=dst, in_=pr, func=AF.Identity, bias=b_l, scale=1.0
                                )
                            else:
                                nc.vector.tensor_scalar(
                                    out=dst, in0=pr, scalar1=b_l, scalar2=None,
                                    op0=mybir.AluOpType.add,
                                )
                            ei += 1
                    q_cols = q_t.rearrange("p (h w) -> p w h", w=WH)
                    k_cols = k_t.rearrange("p (h w) -> p w h", w=WH)

                    # ---- B1: scores/exp/colsum/recip + V^T --------------------
                    es_t = esp.tile([H, WH * H], BF16, tag="es", bufs=1)
                    vt_t = vtp.tile([H, WH, 256], BF16, tag="vt")
                    rt_t = rtp.tile([H, WH], F32, tag="rt")
                    for ch in range(NCH):
                        s_t = ps.tile([H, WC * H], F32, tag="s", bufs=1)
                        for j in range(WC):
                            wl = ch * WC + j
                            nc.tensor.matmul(
                                s_t[:, j * H : (j + 1) * H],
                                k_cols[:, wl, :], q_cols[:, wl, :],
                                start=True, stop=True,
                            )
                        es_ch = es_t[:, ch * WC * H : (ch + 1) * WC * H]
                        nc.scalar.activation(out=es_ch, in_=s_t[:, :], func=AF.Exp)
                        cs_p = ps.tile([H, WC], F32, tag="cs", bufs=1)
                        for j in range(WC):
                            wl = ch * WC + j
                            nc.tensor.matmul(
                                cs_p[:, j : j + 1],
                                es_t[:, wl * H : (wl + 1) * H], invg_t,
                                start=True, stop=True,
                            )
                        nc.vector.reciprocal(
                            out=rt_t[:, ch * WC : (ch + 1) * WC], in_=cs_p
                        )
                        for pair in range(2):
                            vp = ps.tile([H, 512], F32, tag="vtp", bufs=1)
                            for j2 in range(2):
                                wl = ch * WC + pair * 2 + j2
                                for ci in range(2):
                                    nc.tensor.matmul(
                                        vp[:, j2 * 256 : (j2 + 1) * 256],
                                        x_cols[ci][:, half * WH + wl, :],
                                        wvt_t[:, ci, :],
                                        start=(ci == 0), stop=(ci == 1),
                                    )
                            nc.scalar.copy(
                                out=vt_t[:, ch * WC + pair * 2 : ch * WC + pair * 2 + 2, :],
                                in_=vp,
                            )

                    # ---- B2: gamma/colsum as bf16 for the bcast matmul --------
                    rtb_t = rtp.tile([H, WH], BF16, tag="rw")
                    nc.vector.tensor_copy(rtb_t, rt_t)

                    # ---- B3: bcast, U, normalize, residual --------------------
                    for ch in range(NCH):
                        bcp = ps.tile([P, WC * H], F32, tag="bcp", bufs=2)
                        for j in range(WC):
                            wl = ch * WC + j
                            nc.tensor.matmul(
                                bcp[:, j * H : (j + 1) * H],
                                rtb_t[:, wl : wl + 1].to_broadcast([H, P]),
                                idb_t, start=True, stop=True,
                            )
                        bc_t = bctp.tile([P, WC * H], BF16, tag="bc")
                        nc.vector.tensor_copy(bc_t, bcp)
                        bc3 = bc_t.rearrange("p (c h) -> p c h", h=H)
                        for ci in range(2):
                            u_t = ps.tile([P, WC * H], F32, tag="u", bufs=3)
                            u3 = u_t.rearrange("p (c k) -> p c k", k=H)
                            for j in range(WC):
                                wl = ch * WC + j
                                nc.tensor.matmul(
                                    u_t[:, j * H : (j + 1) * H],
                                    vt_t[:, wl, ci * 128 : (ci + 1) * 128],
                                    es_t[:, wl * H : (wl + 1) * H],
                                    start=True, stop=True,
                                )
                            t_t = ttp.tile([P, WC * H], BF16, tag="t")
                            t3 = t_t.rearrange("p (c h) -> p c h", h=H)
                            nc.vector.tensor_mul(t3, u3, bc3)
                            xslice = x_cols[ci][
                                :, half * WH + ch * WC : half * WH + (ch + 1) * WC, :
                            ]
                            fslice = f_cols[ci][
                                :, half * WH + ch * WC : half * WH + (ch + 1) * WC, :
                            ]
                            eng = getattr(nc, TTADD_ENGINE)
                            eng.tensor_add(fslice, t3, xslice)

                for ci in range(2):
                    nc.sync.dma_start(out=out_d.ap()[b, ci], in_=f_ts[ci])
    nc.compile()
    return nc


_NC_CACHE = None


def _get_nc():
    global _NC_CACHE
    if _NC_CACHE is None:
        _NC_CACHE = _build()
    return _NC_CACHE


def kernel(x, Wq, bq, Wk, bk, Wv, bv, gamma):
    x = np.asarray(x, np.float32)
    Wq = np.asarray(Wq, np.float32)
    bq = np.asarray(bq, np.float32)
    Wk = np.asarray(Wk, np.float32)
    bk = np.asarray(bk, np.float32)
    Wv = np.asarray(Wv, np.float32)
    bv = np.asarray(bv, np.float32)
    g = float(np.asarray(gamma, np.float32)[0])

    C = 256
    e = (g * np.linalg.solve(np.eye(C, dtype=np.float64) + g * Wv.astype(np.float64),
                             bv.astype(np.float64))).astype(np.float32)
    xb = (x + e[None, :, None, None]).astype(ml_dtypes.bfloat16)
    xb = np.ascontiguousarray(xb).reshape(16, 2, P, HW)

    cblob = np.zeros((P, 866), np.float32)
    cblob[:, 0:128] = np.stack([Wq[:, :128].T, Wq[:, 128:].T], axis=1).reshape(P, 128)
    cblob[:, 128:256] = np.stack([Wk[:, :128].T, Wk[:, 128:].T], axis=1).reshape(P, 128)
    cblob[:, 256:768] = np.stack([Wv[:, :128].T, Wv[:, 128:].T], axis=1).reshape(P, 512)
    cblob[0:H, 769] = 1.0 / g
    cblob[0:H, 770:866] = np.eye(H, dtype=np.float32)
    cblob = cblob.astype(ml_dtypes.bfloat16)
    bblob = np.zeros((P, 2), np.float32)
    bblob[0:64, 0] = bq - Wq @ e
    bblob[64:128, 0] = bk - Wk @ e
    bblob[0:H, 1] = g

    nc = _get_nc()
    in_maps = []
    for core in range(8):
        in_maps.append({
            "xb": xb[core * B_LOC : (core + 1) * B_LOC],
            "cblob": cblob, "bblob": bblob,
        })
    res = run_bass_kernel_spmd(nc, in_maps, core_ids=list(range(8)))
    outs = [r["out"].reshape(B_LOC, C, H, W) for r in res.results]
    return np.concatenate(outs, axis=0)


def prepared_in_maps(inputs):
    """test-harness helper: the per-core in_maps for a full input dict."""
    import inspect
    sig = ("x", "Wq", "bq", "Wk", "bk", "Wv", "bv", "gamma")
    global _CAPTURE
    _CAPTURE = None
    # rebuild the same host prep by calling kernel body up to run: duplicate code
    x = np.asarray(inputs["x"], np.float32)
    Wq = np.asarray(inputs["Wq"], np.float32); bq = np.asarray(inputs["bq"], np.float32)
    Wk = np.asarray(inputs["Wk"], np.float32); bk = np.asarray(inputs["bk"], np.float32)
    Wv = np.asarray(inputs["Wv"], np.float32); bv = np.asarray(inputs["bv"], np.float32)
    g = float(np.asarray(inputs["gamma"], np.float32)[0])
    C = 256
    e = (g * np.linalg.solve(np.eye(C, dtype=np.float64) + g * Wv.astype(np.float64),
                             bv.astype(np.float64))).astype(np.float32)
    xb = (x + e[None, :, None, None]).astype(ml_dtypes.bfloat16)
    xb = np.ascontiguousarray(xb).reshape(16, 2, P, HW)
    cblob = np.zeros((P, 866), np.float32)
    cblob[:, 0:128] = np.stack([Wq[:, :128].T, Wq[:, 128:].T], axis=1).reshape(P, 128)
    cblob[:, 128:256] = np.stack([Wk[:, :128].T, Wk[:, 128:].T], axis=1).reshape(P, 128)
    cblob[:, 256:768] = np.stack([Wv[:, :128].T, Wv[:, 128:].T], axis=1).reshape(P, 512)
    cblob[0:H, 769] = 1.0 / g
    cblob[0:H, 770:866] = np.eye(H, dtype=np.float32)
    cblob = cblob.astype(ml_dtypes.bfloat16)
    bblob = np.zeros((P, 2), np.float32)
    bblob[0:64, 0] = bq - Wq @ e
    bblob[64:128, 0] = bk - Wk @ e
    bblob[0:H, 1] = g
    return [
        {"xb": xb[c * B_LOC : (c + 1) * B_LOC], "cblob": cblob, "bblob": bblob}
        for c in range(8)
    ]



# revision 2
# speedup vs baseline: 1.1634x; 1.1634x over previous
"""ColAttention TRN2 kernel v2: 8-core data-parallel over batch (2 batches/core).

Math (per batch b, width-column w):
  Q = Wq@x+bq; K = Wk@x+bk; V = Wv@x+bv        (1x1 convs over c)
  S[h,g] = sum_q Q[q,h]K[q,g]; attn = softmax_g(S)
  out = gamma * (attn @ V^T)^T + x

Host folds bv via e = gamma*(I+gamma*Wv)^-1 bv: xb = x+e, bq' = bq-Wq@e,
bk' = bk-Wk@e => device never touches bv; residual add of xb is exact.

v2 design (vs v1): transposed-output scheme.
  - Per 4-column chunk: Q/K projections (N=384), S^T per column,
    exp on ACT, V^T per column, then U^T[h,c] = es_col^T-contracted
    matmul with M=h so the softmax normalizer r[h]=gamma/colsum lands
    on PARTITIONS -> fused (u*r)+xT in ONE scalar_tensor_tensor op.
  - colsum via N=1 matmul reusing es_col as lhsT (no PE broadcast, no
    normalize-multiply pass over es or U).
  - Output written bf16 into the xT tile in place, DMA'd per half,
    upcast to f32 on host. Input DMA'd in two layouts (c-major + h-major).
  - Single flat software-pipelined chunk loop (depth 2) keeps every
    engine continuously fed; all DMA overlapped.
"""
import sys

sys.path.insert(0, "/opt/trn_rl_repo")

import numpy as np
import ml_dtypes

import concourse.bass as bass
import concourse.bacc as bacc
import concourse.mybir as mybir
import concourse.tile as tile
from concourse.bass_utils import run_bass_kernel_spmd

F32 = mybir.dt.float32
BF16 = mybir.dt.bfloat16
AF = mybir.ActivationFunctionType

P = 128
H = 96
W = 96
B_LOC = 2       # batches per core
WH = 48         # columns per w-half
WC = 4          # columns per chunk
NCH = WH // WC  # 12 chunks per half
NG = B_LOC * 2 * NCH  # 48 chunks total per core


def _build():
    nc = bacc.Bacc("TRN2", target_bir_lowering=False, debug=False)

    # inputs: c-major x [b, half, ci, 128, 48*96], h-major xT [b, half, 96, 48*256]
    xc_d = nc.dram_tensor("xc", [B_LOC, 2, 2, P, WH * H], BF16, kind="ExternalInput")
    xt_d = nc.dram_tensor("xt", [B_LOC, 2, H, WH * 256], BF16, kind="ExternalInput")
    cb_d = nc.dram_tensor("cblob", [P, 770], BF16, kind="ExternalInput")
    bb_d = nc.dram_tensor("bblob", [P, 1], F32, kind="ExternalInput")
    out_d = nc.dram_tensor("out", [B_LOC, 2, H, WH * 256], BF16, kind="ExternalOutput")

    with tile.TileContext(nc) as tc:
        import contextlib

        ctx = contextlib.ExitStack()
        with ctx:
            consts = ctx.enter_context(tc.tile_pool(name="consts", bufs=1))
            xcp = ctx.enter_context(tc.tile_pool(name="xcp", bufs=2))
            xtp = ctx.enter_context(tc.tile_pool(name="xtp", bufs=3))
            qkp = ctx.enter_context(tc.tile_pool(name="qkp", bufs=3))
            esp = ctx.enter_context(tc.tile_pool(name="esp", bufs=3))
            vtp = ctx.enter_context(tc.tile_pool(name="vtp", bufs=4))
            rp = ctx.enter_context(tc.tile_pool(name="rp", bufs=3))
            psq = ctx.enter_context(tc.tile_pool(name="psq", bufs=1, space="PSUM"))
            psk = ctx.enter_context(tc.tile_pool(name="psk", bufs=1, space="PSUM"))
            pss = ctx.enter_context(tc.tile_pool(name="pss", bufs=2, space="PSUM"))
            psv = ctx.enter_context(tc.tile_pool(name="psv", bufs=2, space="PSUM"))
            psu = ctx.enter_context(tc.tile_pool(name="psu", bufs=2, space="PSUM"))

            cb_t = consts.tile([P, 770], BF16)
            bb_t = consts.tile([P, 1], F32)
            nc.sync.dma_start(out=cb_t, in_=cb_d.ap())
            nc.sync.dma_start(out=bb_t, in_=bb_d.ap())
            # observers: funnel const-DMA deps into single engine sems
            nc.tensor.ldweights(cb_t[:, 0:64])
            bias_t = consts.tile([P, 1], F32)
            nc.vector.tensor_copy(bias_t, bb_t)
            wq_t = cb_t[:, 0:128].rearrange("p (c m) -> p c m", c=2)     # [128,2,64]
            wk_t = cb_t[:, 128:256].rearrange("p (c m) -> p c m", c=2)   # [128,2,64]
            wvt_t = cb_t[:, 256:768].rearrange("p (c m) -> p c m", c=2)  # [128,2,256]
            bq_t = bias_t[0:64, 0:1]
            bk_t = bias_t[64:128, 0:1]
            invg_t = cb_t[0:H, 769:770]

            # per-chunk state passed across pipeline stages
            xc_tiles = {}   # (b, half) -> tile [128, 2, WH*H]
            xt_tiles = {}   # (b, half) -> tile [H, WH*256]
            qs = {}
            ks = {}
            ess = {}
            vts = {}
            pss_t = {}
            psu_t = {}
            rs = {}

            def bh(g):
                b, r = divmod(g, 2 * NCH)
                half, ch = divmod(r, NCH)
                return b, half, ch

            def load_bh(b, half):
                x_t = xcp.tile([P, 2, WH * H], BF16, tag="xc")
                for ci in range(2):
                    nc.sync.dma_start(out=x_t[:, ci, :], in_=xc_d.ap()[b, half, ci])
                t_t = xtp.tile([H, WH * 256], BF16, tag="xt")
                nc.sync.dma_start(out=t_t, in_=xt_d.ap()[b, half])
                xc_tiles[(b, half)] = x_t
                xt_tiles[(b, half)] = t_t

            def st_proj(g):
                b, half, ch = bh(g)
                if ch == 0 and (b, half) not in xc_tiles:
                    load_bh(b, half)
                # prefetch next half's inputs one chunk into this half
                if ch == 1:
                    nb, nr = divmod(g + NCH, 2 * NCH)
                    nhalf = nr // NCH
                    if nb < B_LOC and (nb, nhalf) not in xc_tiles:
                        load_bh(nb, nhalf)
                x_t = xc_tiles[(b, half)]
                xr = x_t.rearrange("p c (w h) -> p c w h", h=H)
                q_p = psq.tile([64, WC * H], F32, tag="q")
                k_p = psk.tile([64, WC * H], F32, tag="k")
                for ci in range(2):
                    rhs = x_t[:, ci, ch * WC * H : (ch + 1) * WC * H]
                    nc.tensor.matmul(q_p, wq_t[:, ci, :], rhs,
                                     start=(ci == 0), stop=(ci == 1))
                for ci in range(2):
                    rhs = x_t[:, ci, ch * WC * H : (ch + 1) * WC * H]
                    nc.tensor.matmul(k_p, wk_t[:, ci, :], rhs,
                                     start=(ci == 0), stop=(ci == 1))
                # evacs: k on ACT, q on DVE
                q_t = qkp.tile([64, WC * H], BF16, tag="qs")
                k_t = qkp.tile([64, WC * H], BF16, tag="ks")
                nc.scalar.activation(out=k_t, in_=k_p, func=AF.Identity, bias=bk_t)
                nc.vector.tensor_scalar(out=q_t, in0=q_p, scalar1=bq_t, scalar2=None,
                                        op0=mybir.AluOpType.add)
                qs[g], ks[g] = q_t, k_t

            def st_s_vt(g):
                b, half, ch = bh(g)
                x_t = xc_tiles[(b, half)]
                xcols = x_t.rearrange("p c (w h) -> p c w h", h=H)
                q_t, k_t = qs.pop(g), ks.pop(g)
                s_p = pss.tile([H, WC * H + WC], F32, tag="s")
                for j in range(WC):
                    nc.tensor.matmul(
                        s_p[:, j * H : (j + 1) * H],
                        k_t[:, j * H : (j + 1) * H],
                        q_t[:, j * H : (j + 1) * H],
                        start=True, stop=True)
                es_t = esp.tile([H, WC * H], BF16, tag="es")
                nc.scalar.activation(out=es_t, in_=s_p[:, 0 : WC * H], func=AF.Exp)
                ess[g] = es_t
                pss_t[g] = s_p
                # V^T per column pair (independent of S chain)
                vt_pair = []
                for pair in range(2):
                    v_p = psv.tile([H, 512], F32, tag="v")
                    for j2 in range(2):
                        wl = ch * WC + pair * 2 + j2
                        for ci in range(2):
                            nc.tensor.matmul(
                                v_p[:, j2 * 256 : (j2 + 1) * 256],
                                xcols[:, ci, wl, :],
                                wvt_t[:, ci, :],
                                start=(ci == 0), stop=(ci == 1))
                    vt_t = vtp.tile([H, 512], BF16, tag="vt")
                    nc.scalar.copy(out=vt_t, in_=v_p)
                    vt_pair.append(vt_t)
                vts[g] = vt_pair

            def st_u(g):
                es_t = ess.pop(g)
                s_p = pss_t.pop(g)
                vt_pair = vts.pop(g)
                u_ps = []
                for pair in range(2):
                    u_p = psu.tile([H, 512], F32, tag="u")
                    for j2 in range(2):
                        j = pair * 2 + j2
                        nc.tensor.matmul(
                            u_p[:, j2 * 256 : (j2 + 1) * 256],
                            es_t[:, j * H : (j + 1) * H],
                            vt_pair[pair][:, j2 * 256 : (j2 + 1) * 256],
                            start=True, stop=True)
                    u_ps.append(u_p)
                for j in range(WC):
                    nc.tensor.matmul(
                        s_p[:, WC * H + j : WC * H + j + 1],
                        es_t[:, j * H : (j + 1) * H],
                        invg_t,
                        start=True, stop=True)
                r_t = rp.tile([H, WC], F32, tag="r")
                nc.vector.reciprocal(out=r_t, in_=s_p[:, WC * H : WC * H + WC])
                rs[g] = (u_ps, r_t)

            def st_fin(g):
                b, half, ch = bh(g)
                u_ps, r_t = rs.pop(g)
                t_t = xt_tiles[(b, half)]
                tv = t_t.rearrange("p (w c) -> p w c", c=256)
                for j in range(WC):
                    u_p = u_ps[j // 2]
                    u_slice = u_p[:, (j % 2) * 256 : (j % 2 + 1) * 256]
                    dst = tv[:, ch * WC + j, :]
                    nc.vector.scalar_tensor_tensor(
                        out=dst, in0=u_slice, scalar=r_t[:, j : j + 1], in1=dst,
                        op0=mybir.AluOpType.mult, op1=mybir.AluOpType.add)
                if ch == NCH - 1:
                    nc.sync.dma_start(out=out_d.ap()[b, half], in_=t_t)
                    del xc_tiles[(b, half)], xt_tiles[(b, half)]

            # software pipeline, depth 2
            for g in range(NG + 2):
                if g < NG:
                    st_proj(g)
                if 1 <= g < NG + 1:
                    st_s_vt(g - 1)
                if g >= 2:
                    st_u(g - 2)
                    st_fin(g - 2)
    nc.compile()
    return nc


_NC_CACHE = None


def _get_nc():
    global _NC_CACHE
    if _NC_CACHE is None:
        _NC_CACHE = _build()
    return _NC_CACHE


def _prep(x, Wq, bq, Wk, bk, Wv, bv, gamma):
    x = np.asarray(x, np.float32)
    Wq = np.asarray(Wq, np.float32)
    bq = np.asarray(bq, np.float32)
    Wk = np.asarray(Wk, np.float32)
    bk = np.asarray(bk, np.float32)
    Wv = np.asarray(Wv, np.float32)
    bv = np.asarray(bv, np.float32)
    g = float(np.asarray(gamma, np.float32)[0])

    C = 256
    e = (g * np.linalg.solve(np.eye(C, dtype=np.float64) + g * Wv.astype(np.float64),
                             bv.astype(np.float64))).astype(np.float32)
    xb = x + e[None, :, None, None]
    # xc: [16, half, ci, 128, 48, 96]  from (b, c, h, w) -> (b, c, w, h)
    xwh = np.ascontiguousarray(np.transpose(xb, (0, 1, 3, 2)))  # b, c, w, h
    xc = xwh.reshape(16, 2, P, 2, WH, H).transpose(0, 3, 1, 2, 4, 5)
    xc = np.ascontiguousarray(xc).astype(ml_dtypes.bfloat16)
    xc = xc.reshape(16, 2, 2, P, WH * H)
    # xt: [16, half, 96, 48, 256] from (b, h, w, c)
    xhwc = np.ascontiguousarray(np.transpose(xb, (0, 2, 3, 1)))  # b, h, w, c
    xt = xhwc.reshape(16, H, 2, WH, C).transpose(0, 2, 1, 3, 4)
    xt = np.ascontiguousarray(xt).astype(ml_dtypes.bfloat16)
    xt = xt.reshape(16, 2, H, WH * C)

    # blob: 0:128 wq, 128:256 wk, 256:768 wvt, col 769 invg
    blob = np.zeros((P, 770), np.float32)
    blob[:, 0:128] = np.stack([Wq[:, :128].T, Wq[:, 128:].T], axis=1).reshape(P, 128)
    blob[:, 128:256] = np.stack([Wk[:, :128].T, Wk[:, 128:].T], axis=1).reshape(P, 128)
    blob[:, 256:768] = np.stack([Wv.T[:128], Wv.T[128:]], axis=1).reshape(P, 512)
    blob[0:H, 769] = 1.0 / g
    blob = blob.astype(ml_dtypes.bfloat16)

    bblob = np.zeros((P, 1), np.float32)
    bblob[0:64, 0] = bq - Wq @ e
    bblob[64:128, 0] = bk - Wk @ e
    return xc, xt, blob, bblob


def kernel(x, Wq, bq, Wk, bk, Wv, bv, gamma):
    xc, xt, blob, bblob = _prep(x, Wq, bq, Wk, bk, Wv, bv, gamma)
    nc = _get_nc()
    in_maps = []
    for core in range(8):
        in_maps.append({
            "xc": xc[core * B_LOC : (core + 1) * B_LOC],
            "xt": xt[core * B_LOC : (core + 1) * B_LOC],
            "cblob": blob, "bblob": bblob,
        })
    res = run_bass_kernel_spmd(nc, in_maps, core_ids=list(range(8)))
    outs = [r["out"] for r in res.results]
    full = np.concatenate(outs, axis=0)  # [16, 2, 96, 48*256] bf16
    full = full.reshape(16, 2, H, WH, 256).astype(np.float32)
    # (b, half, h, w48, c) -> (b, c, h, w)
    full = full.transpose(0, 4, 2, 1, 3).reshape(16, 256, H, W)
    return np.ascontiguousarray(full)


def prepared_in_maps(inputs):
    xc, xt, blob, bblob = _prep(**inputs)
    return [
        {"xc": xc[c * B_LOC : (c + 1) * B_LOC], "xt": xt[c * B_LOC : (c + 1) * B_LOC],
         "cblob": blob, "bblob": bblob}
        for c in range(8)
    ]


# revision 3
# speedup vs baseline: 1.1900x; 1.0229x over previous
"""ColAttention TRN2 kernel v2: 8-core data-parallel over batch (2 batches/core).

Math (per batch b, width-column w):
  Q = Wq@x+bq; K = Wk@x+bk; V = Wv@x+bv        (1x1 convs over c)
  S[h,g] = sum_q Q[q,h]K[q,g]; attn = softmax_g(S)
  out = gamma * (attn @ V^T)^T + x

Host folds bv via e = gamma*(I+gamma*Wv)^-1 bv: xb = x+e, bq' = bq-Wq@e,
bk' = bk-Wk@e => device never touches bv; residual add of xb is exact.

v2 design (vs v1): transposed-output scheme.
  - Per 4-column chunk: Q/K projections (N=384), S^T per column,
    exp on ACT, V^T per column, then U^T[h,c] = es_col^T-contracted
    matmul with M=h so the softmax normalizer r[h]=gamma/colsum lands
    on PARTITIONS -> fused (u*r)+xT in ONE scalar_tensor_tensor op.
  - colsum via N=1 matmul reusing es_col as lhsT (no PE broadcast, no
    normalize-multiply pass over es or U).
  - Output written bf16 into the xT tile in place, DMA'd per half,
    upcast to f32 on host. Input DMA'd in two layouts (c-major + h-major).
  - Single flat software-pipelined chunk loop (depth 2) keeps every
    engine continuously fed; all DMA overlapped.
"""
import sys

sys.path.insert(0, "/opt/trn_rl_repo")

import numpy as np
import ml_dtypes

import concourse.bass as bass
import concourse.bacc as bacc
import concourse.mybir as mybir
import concourse.tile as tile
from concourse.bass_utils import run_bass_kernel_spmd

F32 = mybir.dt.float32
BF16 = mybir.dt.bfloat16
AF = mybir.ActivationFunctionType

P = 128
H = 96
W = 96
B_LOC = 2       # batches per core
WH = 48         # columns per w-half
WC = 4          # columns per chunk
NCH = WH // WC  # 12 chunks per half
NG = B_LOC * 2 * NCH  # 48 chunks total per core


def _build():
    nc = bacc.Bacc("TRN2", target_bir_lowering=False, debug=False)

    # inputs: c-major x [b, half, ci, 128, 48*96], h-major xT [b, half, 96, 48*256]
    xc_d = nc.dram_tensor("xc", [B_LOC, 2, 2, P, WH * H], BF16, kind="ExternalInput")
    xt_d = nc.dram_tensor("xt", [B_LOC, 2, H, WH * 256], BF16, kind="ExternalInput")
    cb_d = nc.dram_tensor("cblob", [P, 1026], BF16, kind="ExternalInput")
    bb_d = nc.dram_tensor("bblob", [P, 2], F32, kind="ExternalInput")
    out_d = nc.dram_tensor("out", [B_LOC, 2, H, WH * 256], BF16, kind="ExternalOutput")

    with tile.TileContext(nc) as tc:
        import contextlib

        ctx = contextlib.ExitStack()
        with ctx:
            consts = ctx.enter_context(tc.tile_pool(name="consts", bufs=1))
            xcp = ctx.enter_context(tc.tile_pool(name="xcp", bufs=2))
            xtp = ctx.enter_context(tc.tile_pool(name="xtp", bufs=3))
            qkp = ctx.enter_context(tc.tile_pool(name="qkp", bufs=3))
            esp = ctx.enter_context(tc.tile_pool(name="esp", bufs=3))
            vtp = ctx.enter_context(tc.tile_pool(name="vtp", bufs=4))
            rp = ctx.enter_context(tc.tile_pool(name="rp", bufs=3))
            psq = ctx.enter_context(tc.tile_pool(name="psq", bufs=1, space="PSUM"))
            psk = ctx.enter_context(tc.tile_pool(name="psk", bufs=1, space="PSUM"))
            pss = ctx.enter_context(tc.tile_pool(name="pss", bufs=2, space="PSUM"))
            psv = ctx.enter_context(tc.tile_pool(name="psv", bufs=2, space="PSUM"))
            psu = ctx.enter_context(tc.tile_pool(name="psu", bufs=2, space="PSUM"))

            cb_t = consts.tile([P, 1026], BF16)
            bb_t = consts.tile([P, 2], F32)
            nc.sync.dma_start(out=cb_t, in_=cb_d.ap())
            nc.sync.dma_start(out=bb_t, in_=bb_d.ap())
            # observers: funnel const-DMA deps into single engine sems
            nc.tensor.ldweights(cb_t[:, 0:64])
            bias_t = consts.tile([P, 2], F32)
            nc.vector.tensor_copy(bias_t, bb_t)
            # q/k proj weights duplicated over both partition halves (M=128)
            wq_t = cb_t[:, 0:256].rearrange("p (c m) -> p c m", c=2)     # [128,2,128]
            wk_t = cb_t[:, 256:512].rearrange("p (c m) -> p c m", c=2)   # [128,2,128]
            wvt_t = cb_t[:, 512:1024].rearrange("p (c m) -> p c m", c=2)  # [128,2,256]
            bq_t = bias_t[:, 0:1]
            bk_t = bias_t[:, 1:2]
            invg_t = cb_t[0:H, 1025:1026]

            # per-chunk state passed across pipeline stages
            xc_tiles = {}   # (b, half) -> tile [128, 2, WH*H]
            xt_tiles = {}   # (b, half) -> tile [H, WH*256]
            qs = {}
            ks = {}
            ess = {}
            vts = {}
            pss_t = {}
            psu_t = {}
            rs = {}

            def bh(g):
                b, r = divmod(g, 2 * NCH)
                half, ch = divmod(r, NCH)
                return b, half, ch

            def load_bh(b, half, pieces=1):
                # pieces>1 splits each transfer so early chunks' slice deps
                # clear sooner (used for the very first load)
                x_t = xcp.tile([P, 2, WH * H], BF16, tag="xc")
                np_ = WH * H // pieces
                for ci in range(2):
                    for pc in range(pieces):
                        nc.sync.dma_start(
                            out=x_t[:, ci, pc * np_ : (pc + 1) * np_],
                            in_=xc_d.ap()[b, half, ci, :, pc * np_ : (pc + 1) * np_])
                t_t = xtp.tile([H, WH * 256], BF16, tag="xt")
                nt = WH * 256 // pieces
                for pc in range(pieces):
                    nc.sync.dma_start(
                        out=t_t[:, pc * nt : (pc + 1) * nt],
                        in_=xt_d.ap()[b, half, :, pc * nt : (pc + 1) * nt])
                xc_tiles[(b, half)] = x_t
                xt_tiles[(b, half)] = t_t

            def st_proj(g):
                b, half, ch = bh(g)
                if ch == 0 and (b, half) not in xc_tiles:
                    load_bh(b, half, pieces=4 if g == 0 else 1)
                # prefetch next half's inputs one chunk into this half
                if ch == 1:
                    nb, nr = divmod(g + NCH, 2 * NCH)
                    nhalf = nr // NCH
                    if nb < B_LOC and (nb, nhalf) not in xc_tiles:
                        load_bh(nb, nhalf)
                x_t = xc_tiles[(b, half)]
                q_p = psq.tile([P, WC * H], F32, tag="q")
                k_p = psk.tile([P, WC * H], F32, tag="k")
                for ci in range(2):
                    rhs = x_t[:, ci, ch * WC * H : (ch + 1) * WC * H]
                    nc.tensor.matmul(q_p, wq_t[:, ci, :], rhs,
                                     start=(ci == 0), stop=(ci == 1))
                for ci in range(2):
                    rhs = x_t[:, ci, ch * WC * H : (ch + 1) * WC * H]
                    nc.tensor.matmul(k_p, wk_t[:, ci, :], rhs,
                                     start=(ci == 0), stop=(ci == 1))
                # evacs: k on ACT, q on DVE (q/k live duplicated in both halves)
                q_t = qkp.tile([P, WC * H], BF16, tag="qs")
                k_t = qkp.tile([P, WC * H], BF16, tag="ks")
                nc.scalar.activation(out=k_t, in_=k_p, func=AF.Identity, bias=bk_t)
                nc.vector.tensor_scalar(out=q_t, in0=q_p, scalar1=bq_t, scalar2=None,
                                        op0=mybir.AluOpType.add)
                qs[g], ks[g] = q_t, k_t

            def st_s_vt(g):
                b, half, ch = bh(g)
                x_t = xc_tiles[(b, half)]
                xcols = x_t.rearrange("p c (w h) -> p c w h", h=H)
                q_t, k_t = qs.pop(g), ks.pop(g)
                s_p = pss.tile([H, WC * H + WC], F32, tag="s")
                for j in range(WC):
                    # alternate PE row-groups (K=64): even cols read the 0:64
                    # copy, odd cols the 64:128 copy -> T0/T8 concurrency
                    base = 0  # bisect: base-64 path caused device fault
                    nc.tensor.matmul(
                        s_p[:, j * H : (j + 1) * H],
                        k_t[base : base + 64, j * H : (j + 1) * H],
                        q_t[base : base + 64, j * H : (j + 1) * H],
                        start=True, stop=True)
                es_t = esp.tile([H, WC * H], BF16, tag="es")
                nc.scalar.activation(out=es_t, in_=s_p[:, 0 : WC * H], func=AF.Exp)
                ess[g] = es_t
                pss_t[g] = s_p
                # V^T per column pair (independent of S chain)
                vt_pair = []
                for pair in range(2):
                    v_p = psv.tile([H, 512], F32, tag="v")
                    for j2 in range(2):
                        wl = ch * WC + pair * 2 + j2
                        for ci in range(2):
                            nc.tensor.matmul(
                                v_p[:, j2 * 256 : (j2 + 1) * 256],
                                xcols[:, ci, wl, :],
                                wvt_t[:, ci, :],
                                start=(ci == 0), stop=(ci == 1))
                    vt_t = vtp.tile([H, 512], BF16, tag="vt")
                    nc.scalar.copy(out=vt_t, in_=v_p)
                    vt_pair.append(vt_t)
                vts[g] = vt_pair

            def st_u(g):
                es_t = ess.pop(g)
                s_p = pss_t.pop(g)
                vt_pair = vts.pop(g)
                u_ps = []
                for pair in range(2):
                    u_p = psu.tile([H, 512], F32, tag="u")
                    for j2 in range(2):
                        j = pair * 2 + j2
                        nc.tensor.matmul(
                            u_p[:, j2 * 256 : (j2 + 1) * 256],
                            es_t[:, j * H : (j + 1) * H],
                            vt_pair[pair][:, j2 * 256 : (j2 + 1) * 256],
                            start=True, stop=True)
                    u_ps.append(u_p)
                for j in range(WC):
                    nc.tensor.matmul(
                        s_p[:, WC * H + j : WC * H + j + 1],
                        es_t[:, j * H : (j + 1) * H],
                        invg_t,
                        start=True, stop=True)
                r_t = rp.tile([H, WC], F32, tag="r")
                nc.vector.reciprocal(out=r_t, in_=s_p[:, WC * H : WC * H + WC])
                rs[g] = (u_ps, r_t)

            def st_fin(g):
                b, half, ch = bh(g)
                u_ps, r_t = rs.pop(g)
                t_t = xt_tiles[(b, half)]
                tv = t_t.rearrange("p (w c) -> p w c", c=256)
                for j in range(WC):
                    u_p = u_ps[j // 2]
                    u_slice = u_p[:, (j % 2) * 256 : (j % 2 + 1) * 256]
                    dst = tv[:, ch * WC + j, :]
                    nc.vector.scalar_tensor_tensor(
                        out=dst, in0=u_slice, scalar=r_t[:, j : j + 1], in1=dst,
                        op0=mybir.AluOpType.mult, op1=mybir.AluOpType.add)
                if ch == NCH // 2 - 1:
                    hw2 = (WH // 2) * 256
                    nc.sync.dma_start(out=out_d.ap()[b, half, :, 0:hw2],
                                      in_=t_t[:, 0:hw2])
                elif ch == NCH - 1:
                    hw2 = (WH // 2) * 256
                    nc.sync.dma_start(out=out_d.ap()[b, half, :, hw2:],
                                      in_=t_t[:, hw2:])
                    del xc_tiles[(b, half)], xt_tiles[(b, half)]

            # software pipeline, depth 2
            for g in range(NG + 2):
                if g < NG:
                    st_proj(g)
                if 1 <= g < NG + 1:
                    st_s_vt(g - 1)
                if g >= 2:
                    st_u(g - 2)
                    st_fin(g - 2)
    nc.compile()
    return nc


_NC_CACHE = None


def _get_nc():
    global _NC_CACHE
    if _NC_CACHE is None:
        _NC_CACHE = _build()
    return _NC_CACHE


def _prep(x, Wq, bq, Wk, bk, Wv, bv, gamma):
    x = np.asarray(x, np.float32)
    Wq = np.asarray(Wq, np.float32)
    bq = np.asarray(bq, np.float32)
    Wk = np.asarray(Wk, np.float32)
    bk = np.asarray(bk, np.float32)
    Wv = np.asarray(Wv, np.float32)
    bv = np.asarray(bv, np.float32)
    g = float(np.asarray(gamma, np.float32)[0])

    C = 256
    e = (g * np.linalg.solve(np.eye(C, dtype=np.float64) + g * Wv.astype(np.float64),
                             bv.astype(np.float64))).astype(np.float32)
    xb = x + e[None, :, None, None]
    # xc: [16, half, ci, 128, 48, 96]  from (b, c, h, w) -> (b, c, w, h)
    xwh = np.ascontiguousarray(np.transpose(xb, (0, 1, 3, 2)))  # b, c, w, h
    xc = xwh.reshape(16, 2, P, 2, WH, H).transpose(0, 3, 1, 2, 4, 5)
    xc = np.ascontiguousarray(xc).astype(ml_dtypes.bfloat16)
    xc = xc.reshape(16, 2, 2, P, WH * H)
    # xt: [16, half, 96, 48, 256] from (b, h, w, c)
    xhwc = np.ascontiguousarray(np.transpose(xb, (0, 2, 3, 1)))  # b, h, w, c
    xt = xhwc.reshape(16, H, 2, WH, C).transpose(0, 2, 1, 3, 4)
    xt = np.ascontiguousarray(xt).astype(ml_dtypes.bfloat16)
    xt = xt.reshape(16, 2, H, WH * C)

    # blob: 0:256 wq(dup M=128), 256:512 wk(dup), 512:1024 wvt, col 1025 invg
    blob = np.zeros((P, 1026), np.float32)
    wqd = [np.concatenate([Wq[:, s].T, Wq[:, s].T], axis=1)
           for s in (slice(0, 128), slice(128, 256))]  # each [128, 128]
    wkd = [np.concatenate([Wk[:, s].T, Wk[:, s].T], axis=1)
           for s in (slice(0, 128), slice(128, 256))]
    blob[:, 0:256] = np.stack(wqd, axis=1).reshape(P, 256)
    blob[:, 256:512] = np.stack(wkd, axis=1).reshape(P, 256)
    blob[:, 512:1024] = np.stack([Wv.T[:128], Wv.T[128:]], axis=1).reshape(P, 512)
    blob[0:H, 1025] = 1.0 / g
    blob = blob.astype(ml_dtypes.bfloat16)

    bqe = bq - Wq @ e
    bke = bk - Wk @ e
    bblob = np.zeros((P, 2), np.float32)
    bblob[0:64, 0] = bqe
    bblob[64:128, 0] = bqe
    bblob[0:64, 1] = bke
    bblob[64:128, 1] = bke
    return xc, xt, blob, bblob


def kernel(x, Wq, bq, Wk, bk, Wv, bv, gamma):
    xc, xt, blob, bblob = _prep(x, Wq, bq, Wk, bk, Wv, bv, gamma)
    nc = _get_nc()
    in_maps = []
    for core in range(8):
        in_maps.append({
            "xc": xc[core * B_LOC : (core + 1) * B_LOC],
            "xt": xt[core * B_LOC : (core + 1) * B_LOC],
            "cblob": blob, "bblob": bblob,
        })
    res = run_bass_kernel_spmd(nc, in_maps, core_ids=list(range(8)))
    outs = [r["out"] for r in res.results]
    full = np.concatenate(outs, axis=0)  # [16, 2, 96, 48*256] bf16
    full = full.reshape(16, 2, H, WH, 256).astype(np.float32)
    # (b, half, h, w48, c) -> (b, c, h, w)
    full = full.transpose(0, 4, 2, 1, 3).reshape(16, 256, H, W)
    return np.ascontiguousarray(full)


def prepared_in_maps(inputs):
    xc, xt, blob, bblob = _prep(**inputs)
    return [
        {"xc": xc[c * B_LOC : (c + 1) * B_LOC], "xt": xt[c * B_LOC : (c + 1) * B_LOC],
         "cblob": blob, "bblob": bblob}
        for c in range(8)
    ]
